# revision 2
# baseline (speedup 1.0000x reference)
"""Trainium2 Bass kernel v4: 3-layer GRU (B=512,T=512,D=22,H=64) + MLP head.

Beyond v3 (step-interleaved chains, bf16 matmuls):
  - The next step's h-side matmuls are decomposed as W*h' = W*q + W*u with
    q = h - s*h (Pool, off-chain, ready during tanh) and u = s*n (DVE).
    The critical chain therefore ends ...tanh -> u -> [W*u matmuls] -> sigma'
    without waiting for h' to materialize; h' = q + u is computed off-chain
    for the seq tile (next layer's input + p/q of the next step).
  - b_hn rides a ones row on the q tiles (whn lhsT [65,64]); the u-side n
    matmul uses whn[0:64] so the bias is applied exactly once.
  - Layer stagger is 8 steps so a layer's chunk-head x-matmuls are emitted
    after the previous layer finished writing that chunk's seq tile.
"""
import numpy as np
from contextlib import ExitStack

B, T, D_IN, H = 512, 512, 22, 64
NCORES = 8
BL = B // NCORES
CH = 4
CB = CH * BL              # 256
EPS = 1e-5
LAG = 8                   # per-layer stagger in steps

_PROGRAM_CACHE = {}
NA_ON_POOL = False
HP_ON_POOL = False
OPLOG = {}   # instruction name -> (stage, layer, step) when LOG_OPS is set
LOG_OPS = False


def _log(inst, stage, l, s):
    if LOG_OPS and inst is not None:
        OPLOG[inst.ins.name] = (stage, l, s)


def _np32(a):
    return np.ascontiguousarray(np.asarray(a), dtype=np.float32)


def _bf16(a):
    import ml_dtypes
    return np.ascontiguousarray(np.asarray(a, dtype=np.float32).astype(ml_dtypes.bfloat16))


def _prep_weights(inp):
    w = {}
    for l, din in enumerate([D_IN, H, H]):
        w_ih = _np32(inp[f"w_ih{l}"])
        w_hh = _np32(inp[f"w_hh{l}"])
        b_ih = _np32(inp[f"b_ih{l}"])
        b_hh = _np32(inp[f"b_hh{l}"])
        r, z, n = slice(0, H), slice(H, 2 * H), slice(2 * H, 3 * H)

        wxrz = np.zeros((din + 1, 2 * H), np.float32)
        wxrz[:din, 0:H] = -w_ih[z].T
        wxrz[:din, H:2 * H] = w_ih[r].T
        wxrz[din, 0:H] = -(b_ih[z] + b_hh[z])
        wxrz[din, H:2 * H] = b_ih[r] + b_hh[r]
        if l == 0:
            w["wxrz0"] = _bf16(wxrz)
        else:
            w[f"wxrz{l}"] = _bf16(wxrz[:din])
            w[f"wxrzb{l}"] = _bf16(wxrz[din:din + 1])

        wxn = np.zeros((din + 1, H), np.float32)
        wxn[:din] = w_ih[n].T
        wxn[din] = b_ih[n]
        if l == 0:
            w["wxn0"] = _bf16(wxn)
        else:
            w[f"wxn{l}"] = _bf16(wxn[:din])
            w[f"wxnb{l}"] = _bf16(wxn[din:din + 1])

        w[f"whrz{l}"] = _bf16(np.concatenate([-w_hh[z].T, w_hh[r].T], axis=1))

        w[f"whn{l}"] = _bf16(w_hh[n].T)
        w[f"whnb{l}"] = _bf16(b_hh[n][None, :])

    def fold_bn(wf, bf, g, b_, m, v):
        s = g / np.sqrt(v + EPS)
        return wf * s[:, None], (bf - m) * s + b_

    w1, b1 = fold_bn(_np32(inp["fc1_w"]), _np32(inp["fc1_b"]), _np32(inp["bn1_g"]),
                     _np32(inp["bn1_b"]), _np32(inp["bn1_m"]), _np32(inp["bn1_v"]))
    w2, b2 = fold_bn(_np32(inp["fc2_w"]), _np32(inp["fc2_b"]), _np32(inp["bn2_g"]),
                     _np32(inp["bn2_b"]), _np32(inp["bn2_m"]), _np32(inp["bn2_v"]))
    w3, b3 = _np32(inp["fc3_w"]), _np32(inp["fc3_b"])

    fc1 = np.zeros((H + 1, 54), np.float32)
    fc1[:H] = w1.T
    fc1[H] = b1
    fc2 = np.zeros((55, 44), np.float32)
    fc2[:54] = w2.T
    fc2[54] = b2
    fc3 = np.zeros((45, 4), np.float32)
    fc3[:44] = w3.T
    fc3[44] = b3
    w["fc1"] = _bf16(fc1[:H])
    w["fc1b"] = _bf16(fc1[H:H + 1])
    w["fc2"] = _bf16(fc2[:54])
    w["fc2b"] = _bf16(fc2[54:55])
    w["fc3"] = _bf16(fc3[:44])
    w["fc3b"] = _bf16(fc3[44:45])
    return w


def _prep_x_core(x_core):
    t = x_core.shape[2]
    xt = np.empty((D_IN + 1, t * BL), np.float32)
    xt[:D_IN] = _np32(x_core).transpose(1, 2, 0).reshape(D_IN, t * BL)
    xt[D_IN] = 1.0
    return _bf16(xt)


def _build(t_steps):
    import concourse.bacc as bacc
    import concourse.tile as tile
    from concourse import mybir

    f32 = mybir.dt.float32
    bf16 = mybir.dt.bfloat16
    AF = mybir.ActivationFunctionType
    ALU = mybir.AluOpType
    ts = __import__("concourse.bass", fromlist=["ts"]).ts

    nch = t_steps // CH
    nc = bacc.Bacc("TRN2", target_bir_lowering=False, debug=False)

    xt = nc.dram_tensor("xt", [D_IN + 1, t_steps * BL], bf16, kind="ExternalInput").ap()
    wd = {}
    for l in range(3):
        if l == 0:
            wd["wxrz0"] = nc.dram_tensor("wxrz0", [D_IN + 1, 2 * H], bf16, kind="ExternalInput").ap()
            wd["wxn0"] = nc.dram_tensor("wxn0", [D_IN + 1, H], bf16, kind="ExternalInput").ap()
        else:
            wd[f"wxrz{l}"] = nc.dram_tensor(f"wxrz{l}", [H, 2 * H], bf16, kind="ExternalInput").ap()
            wd[f"wxrzb{l}"] = nc.dram_tensor(f"wxrzb{l}", [1, 2 * H], bf16, kind="ExternalInput").ap()
            wd[f"wxn{l}"] = nc.dram_tensor(f"wxn{l}", [H, H], bf16, kind="ExternalInput").ap()
            wd[f"wxnb{l}"] = nc.dram_tensor(f"wxnb{l}", [1, H], bf16, kind="ExternalInput").ap()
        wd[f"whrz{l}"] = nc.dram_tensor(f"whrz{l}", [H, 2 * H], bf16, kind="ExternalInput").ap()
        wd[f"whn{l}"] = nc.dram_tensor(f"whn{l}", [H, H], bf16, kind="ExternalInput").ap()
        wd[f"whnb{l}"] = nc.dram_tensor(f"whnb{l}", [1, H], bf16, kind="ExternalInput").ap()
    wd["fc1"] = nc.dram_tensor("fc1", [H, 54], bf16, kind="ExternalInput").ap()
    wd["fc1b"] = nc.dram_tensor("fc1b", [1, 54], bf16, kind="ExternalInput").ap()
    wd["fc2"] = nc.dram_tensor("fc2", [54, 44], bf16, kind="ExternalInput").ap()
    wd["fc2b"] = nc.dram_tensor("fc2b", [1, 44], bf16, kind="ExternalInput").ap()
    wd["fc3"] = nc.dram_tensor("fc3", [44, 4], bf16, kind="ExternalInput").ap()
    wd["fc3b"] = nc.dram_tensor("fc3b", [1, 4], bf16, kind="ExternalInput").ap()
    y = nc.dram_tensor("y", [4, BL], f32, kind="ExternalOutput").ap()

    with tile.TileContext(nc) as tc, ExitStack() as ctx:
        const = ctx.enter_context(tc.tile_pool(name="const", bufs=1))
        xpool = ctx.enter_context(tc.tile_pool(name="xpool", bufs=2))
        seqp = [ctx.enter_context(tc.tile_pool(name=f"seq{l}", bufs=2))
                for l in range(3)]
        rzp = [ctx.enter_context(tc.tile_pool(name=f"rz{l}", bufs=1, space="PSUM"))
               for l in range(3)]
        hnp = [ctx.enter_context(tc.tile_pool(name=f"hnp{l}", bufs=1, space="PSUM"))
               for l in range(3)]
        scrp = ctx.enter_context(tc.tile_pool(name="scrp", bufs=1, space="PSUM"))
        inbp = ctx.enter_context(tc.tile_pool(name="inbp", bufs=2))
        mlpp = ctx.enter_context(tc.tile_pool(name="mlpp", bufs=1, space="PSUM"))
        sigp = ctx.enter_context(tc.tile_pool(name="sigp", bufs=6))
        qp = ctx.enter_context(tc.tile_pool(name="qp", bufs=4))
        gp = ctx.enter_context(tc.tile_pool(name="gates", bufs=6))
        mlps = ctx.enter_context(tc.tile_pool(name="mlps", bufs=1))

        ws = {}
        for name, ap in wd.items():
            wt = const.tile(list(ap.shape), ap.dtype, name=name)
            nc.sync.dma_start(out=wt, in_=ap)
            ws[name] = wt

        zt = const.tile([H, BL], bf16, name="zt")
        nc.vector.memset(zt[:, :], 0.0)
        ones1 = const.tile([1, BL], bf16, name="ones1")
        nc.vector.memset(ones1[:, :], 1.0)
        onesC = const.tile([1, CB], bf16, name="onesC")
        nc.vector.memset(onesC[:, :], 1.0)

        rz = []
        _hn = []
        for l in range(3):
            rz.append(rzp[l].tile([2 * H, 2 * CB], f32, name=f"rzb{l}"))
            _hn.append(hnp[l].tile([H, CB], f32, name=f"hnb{l}"))
        # i_n goes psum-scratch -> SBUF per chunk (one shared scratch bank;
        # start=True pending-zeroes a 2KB zero-region per partition range, so
        # hn slices must not share a bank with the i_n matmuls).
        scr = scrp.tile([H, CB], f32, name="scr")
        inb = [{} for _ in range(3)]     # (l, c) -> sbuf i_n chunk tile

        seq_tiles = [{} for _ in range(3)]
        x_tiles = {}
        st = [{} for _ in range(3)]

        def get_seq(l, c):
            if c not in seq_tiles[l]:
                tl = seqp[l].tile([H, CB], bf16, name=f"sq{l}")
                seq_tiles[l][c] = tl
                seq_tiles[l].pop(c - 2, None)
            return seq_tiles[l][c]

        def hprev_ap(l, c, j):
            if c == 0 and j == 0:
                return zt
            if j == 0:
                return seq_tiles[l][c - 1][:, ts(CH - 1, BL)]
            return seq_tiles[l][c][:, ts(j - 1, BL)]
        # (all hprev APs are [H, *] now)

        def rz_slice(l, s):
            c, j = divmod(s, CH)
            half = (c % 2) * CB
            return rz[l][:, half + j * BL:half + (j + 1) * BL]

        def hn_slice(l, s):
            j = s % CH
            return _hn[l][:, j * BL:(j + 1) * BL]

        def prefetch_x(c):
            if c < nch and c not in x_tiles:
                xc = xpool.tile([D_IN + 1, CB], bf16, name="xc")
                nc.sync.dma_start(out=xc, in_=xt[:, c * CB:(c + 1) * CB])
                x_tiles[c] = xc
                x_tiles.pop(c - 2, None)

        def head_rz(l, c):
            half = (c % 2) * CB
            if l == 0:
                prefetch_x(c)
                prefetch_x(c + 1)
                src = x_tiles[c]
                nc.tensor.matmul(rz[l][:, half:half + CB], ws["wxrz0"],
                                 src[0:D_IN + 1, :], start=True, stop=False,
                                 skip_group_check=True)
            else:
                src = get_seq(l - 1, c)
                nc.tensor.matmul(rz[l][:, half:half + CB], ws[f"wxrz{l}"],
                                 src, start=True, stop=False,
                                 skip_group_check=True)
                nc.tensor.matmul(rz[l][:, half:half + CB],
                                 ws[f"wxrzb{l}"], onesC,
                                 start=False, stop=False, skip_group_check=True)
            # bias-A: open this chunk's hn region, writing b_hn into slice 0.
            # (Its start=True pending-zeroes the whole bank; its own write
            # clears slice 0, head_n's bias-B covers slices 1:3.)
            nc.tensor.matmul(_hn[l][:, 0:BL], ws[f"whnb{l}"], ones1,
                             start=True, stop=False, skip_group_check=True)
            if c == 0:
                nc.tensor.matmul(rz_slice(l, 0), ws[f"whrz{l}"], zt,
                                 start=False, stop=True, skip_group_check=True)

        def head_n(l, c):
            # i_n region is single-buffered: this must be emitted AFTER the
            # last na read of chunk c-1 (stage_a(c-1, 3)).
            if l == 0:
                src = x_tiles[c]
                nc.tensor.matmul(scr, ws["wxn0"],
                                 src[0:D_IN + 1, :], start=True, stop=True,
                                 skip_group_check=True)
            else:
                src = get_seq(l - 1, c)
                nc.tensor.matmul(scr, ws[f"wxn{l}"],
                                 src, start=True, stop=False,
                                 skip_group_check=True)
                nc.tensor.matmul(scr, ws[f"wxnb{l}"],
                                 onesC, start=False, stop=True,
                                 skip_group_check=True)
            nc.tensor.matmul(_hn[l][:, BL:CH * BL], ws[f"whnb{l}"],
                             onesC[:, 0:(CH - 1) * BL], start=False, stop=False,
                             skip_group_check=True)
            it = inbp.tile([H, CB], f32, name=f"inb{l}")
            nc.vector.tensor_scalar_add(it, scr, 0.0)
            inb[l][c] = it
            inb[l].pop(c - 2, None)
            if c == 0:
                nc.tensor.matmul(hn_slice(l, 0), ws[f"whn{l}"], zt,
                                 start=False, stop=True, skip_group_check=True)

        def stage_a(l, c, j):
            s = c * CH + j
            sig = sigp.tile([2 * H, BL], f32, name=f"sig{l}")
            _log(nc.scalar.activation(sig, rz_slice(l, s), AF.Sigmoid), "sigma", l, s)
            m1 = gp.tile([H, BL], f32, name=f"m1_{l}")
            _log(nc.vector.tensor_mul(m1, sig[H:2 * H, :], hn_slice(l, s)), "m1", l, s)
            na = gp.tile([H, BL], f32, name=f"na_{l}")
            _log(nc.vector.tensor_add(na, m1, inb[l][c][:, ts(j, BL)]), "na", l, s)
            hp = hprev_ap(l, c, j)
            p = gp.tile([H, BL], f32, name=f"p_{l}")
            _log(nc.gpsimd.tensor_mul(p, sig[0:H, :], hp), "p", l, s)
            q = qp.tile([H, BL], bf16, name=f"q_{l}")
            _log(nc.gpsimd.tensor_sub(q, hp, p), "q", l, s)
            if s + 1 < t_steps:
                _log(nc.tensor.matmul(rz_slice(l, s + 1), ws[f"whrz{l}"], q,
                                 start=False, stop=False, skip_group_check=True), "mmrzq", l, s)
                _log(nc.tensor.matmul(hn_slice(l, s + 1), ws[f"whn{l}"], q,
                                 start=False, stop=False, skip_group_check=True), "mmnq", l, s)
            st[l]["sig"], st[l]["na"], st[l]["q"] = sig, na, q

        def stage_b(l, c, j):
            s = c * CH + j
            sig, na, q = st[l]["sig"], st[l]["na"], st[l]["q"]
            nt = gp.tile([H, BL], f32, name=f"nt_{l}")
            _log(nc.scalar.activation(nt, na, AF.Tanh), "tanh", l, s)
            u = gp.tile([H, BL], bf16, name=f"u_{l}")
            _log(nc.vector.tensor_mul(u, sig[0:H, :], nt), "u", l, s)
            if s + 1 < t_steps:
                _log(nc.tensor.matmul(rz_slice(l, s + 1), ws[f"whrz{l}"], u,
                                 start=False, stop=True, skip_group_check=True), "mmrzu", l, s)
                _log(nc.tensor.matmul(hn_slice(l, s + 1), ws[f"whn{l}"], u,
                                 start=False, stop=True, skip_group_check=True), "mmnu", l, s)
            dst = get_seq(l, c)
            _log(nc.vector.tensor_add(dst[:, ts(j, BL)], q, u), "hprime", l, s)

        # ---- schedule ----
        K = t_steps + 2 * LAG
        prev = [None] * 3

        def active(l, k):
            s = k - LAG * l
            return divmod(s, CH) if 0 <= s < t_steps else None

        for k in range(K + 1):
            for l in range(3):
                if prev[l]:
                    stage_b(l, *prev[l])
                cur = active(l, k) if k < K else None
                if cur:
                    s = k - LAG * l
                    if s == 0:
                        head_rz(l, 0)
                        head_n(l, 0)
                    nxt = (s % CH == CH - 1 and (s + 1) // CH < nch)
                    if nxt:
                        head_rz(l, (s + 1) // CH)
                    stage_a(l, *cur)
                    if nxt:
                        head_n(l, (s + 1) // CH)
                prev[l] = cur

        # ---- MLP head ----
        hlast = seq_tiles[2][nch - 1][:, ts(CH - 1, BL)]
        pm1 = mlpp.tile([64, 3 * BL], f32, name="pmlp")
        nc.tensor.matmul(pm1[0:54, 0:BL], ws["fc1"], hlast,
                         start=True, stop=False, skip_group_check=True)
        nc.tensor.matmul(pm1[0:54, 0:BL], ws["fc1b"], ones1,
                         start=False, stop=True, skip_group_check=True)
        y1 = mlps.tile([54, BL], bf16, name="y1")
        nc.vector.tensor_scalar_max(y1, pm1[0:54, 0:BL], 0.0)
        nc.tensor.matmul(pm1[0:44, BL:2 * BL], ws["fc2"], y1,
                         start=True, stop=False, skip_group_check=True)
        nc.tensor.matmul(pm1[0:44, BL:2 * BL], ws["fc2b"], ones1,
                         start=False, stop=True, skip_group_check=True)
        y2 = mlps.tile([44, BL], bf16, name="y2")
        nc.vector.tensor_scalar_max(y2, pm1[0:44, BL:2 * BL], 0.0)
        nc.tensor.matmul(pm1[0:4, 2 * BL:3 * BL], ws["fc3"], y2,
                         start=True, stop=False, skip_group_check=True)
        nc.tensor.matmul(pm1[0:4, 2 * BL:3 * BL], ws["fc3b"], ones1,
                         start=False, stop=True, skip_group_check=True)
        yo = mlps.tile([4, BL], f32, name="yo")
        nc.vector.tensor_scalar_add(yo, pm1[0:4, 2 * BL:3 * BL], 0.0)
        nc.sync.dma_start(out=y, in_=yo)

    nc.compile()
    return nc


def get_program(t_steps=T):
    if t_steps not in _PROGRAM_CACHE:
        _PROGRAM_CACHE[t_steps] = _build(t_steps)
    return _PROGRAM_CACHE[t_steps]


def make_in_maps(inputs, t_steps=T):
    x = _np32(inputs["x"])
    w = _prep_weights(inputs)
    in_maps = []
    for c in range(NCORES):
        m = dict(w)
        m["xt"] = _prep_x_core(x[c * BL:(c + 1) * BL, :, :t_steps])
        in_maps.append(m)
    return in_maps


def kernel(**inputs) -> np.ndarray:
    from concourse.bass_utils import run_bass_kernel_spmd

    nc = get_program(T)
    in_maps = make_in_maps(inputs, T)
    res = run_bass_kernel_spmd(nc, in_maps, list(range(NCORES)))
    out = np.empty((B, 4), np.float32)
    for c in range(NCORES):
        out[c * BL:(c + 1) * BL] = res.results[c]["y"].T
    return out
